# revision 3
# baseline (speedup 1.0000x reference)
"""Trainium2 Bass kernel for nn_Encoder_60318520705555 (DGCNN-style encoder).

Sharding: data-parallel over batch B=8 across 8 NeuronCores (1 batch element
per core); BN batch statistics are all-reduced across cores (6 tiny
AllReduces). Everything else is core-local.

Self-contained: hardcodes shapes (B=8, N=2048, K=16, channel sizes).
"""

import sys

sys.path.insert(0, "/opt/trn_rl_repo")

import numpy as np

import bass_rust
import concourse.bass as bass
import concourse.mybir as mybir
import concourse.tile as tile
from concourse.bass import IndirectOffsetOnAxis
from concourse.bass_utils import run_bass_kernel_spmd
from concourse.masks import make_identity

F32 = mybir.dt.float32
U32 = mybir.dt.uint32
AF = mybir.ActivationFunctionType
AX = mybir.AxisListType
OP = mybir.AluOpType

N_CORES = 8
B = 8
N = 2048
KNN = 16
NB = N // 128  # row blocks
BN_EPS = 1e-5
NEG = -1.0e30
INV_M = 1.0 / (B * N)  # BN mean divisor (global batch)

# conv layer channel sizes
C1_IN, C1_OUT = 12, 64
C2_OUT, C3_OUT = 64, 64
G1_OUT, G2_OUT = 128, 1024
C4_OUT = 512


def ts(i, s):
    return slice(i * s, (i + 1) * s)


def split_drain_waits(nc, limit=1):
    """walrus core_v3 codegen rejects instructions carrying more than one
    sync wait; hoist excess waits onto single-wait NoOp carriers just
    before the instruction (engine streams are in-order, so this is
    semantically equivalent)."""
    for f in nc.m.functions:
        for bb in f.blocks:
            out = []
            changed = False
            for inst in bb.instructions:
                si = inst.sync_info
                if si is not None and len(si.on_wait) > limit:
                    waits = list(si.on_wait)
                    chunks = [waits[i : i + limit] for i in range(0, len(waits), limit)]
                    for j, ch in enumerate(chunks[:-1]):
                        d = mybir.InstNoOp(name=f"{inst.name}-sw{j}", engine=inst.engine)
                        d.sync_info = bass_rust.SyncInfo(on_wait=ch, on_update=[])
                        nc.register_instruction(d, overwrite=True)
                        out.append(d)
                    si.on_wait = chunks[-1]
                    inst.sync_info = si
                    changed = True
                out.append(inst)
            if changed:
                bb.instructions = out


DEBUG = False


def build_program():
    nc = bass.Bass()

    # ---- I/O declarations (per-core shapes; host prepares the layouts) ----
    inp = {}

    def din(name, shape):
        inp[name] = nc.dram_tensor(name, list(shape), F32, kind="ExternalInput")
        return inp[name]

    din("Lt1", (4, N))      # [2*x^T; ones]
    din("Rt1", (4, N))      # [x^T; -aa]
    din("xpad", (N, 4))     # x padded to 4 cols (gather table)
    din("W1T", (C1_IN, C1_OUT))
    din("W2T", (C1_OUT, C2_OUT))
    din("W3T", (C2_OUT, C3_OUT))
    din("Wg1T", (C3_OUT, G1_OUT))
    din("Wg2T", (G1_OUT, G2_OUT))
    din("W4Tp", (128, 8 * C4_OUT))  # K-chunk j at cols [512j:512j+512]
    for nm, c in [("b1", 64), ("gm1", 64), ("bt1", 64), ("b2", 64), ("gm2", 64),
                  ("bt2", 64), ("b3", 64), ("gm3", 64), ("bt3", 64),
                  ("bg1", 128), ("gmg1", 128), ("btg1", 128)]:
        din(nm, (c, 1))
    # 1024-channel vectors as (128, 8): col j = channels [128j, 128j+128)
    for nm in ("bg2", "gmg2", "btg2"):
        din(nm, (128, 8))
    # 512-channel vectors as (128, 4)
    for nm in ("b4", "gm4", "bt4"):
        din(nm, (128, 4))

    out_t = nc.dram_tensor("out", [4, 128], F32, kind="ExternalOutput")
    dbg_t = nc.dram_tensor("dbg", [128, 64], F32, kind="ExternalOutput") if DEBUG else None
    dbg2_t = nc.dram_tensor("dbg2", [128, 4096], F32, kind="ExternalOutput") if DEBUG else None

    with tile.TileContext(nc) as tc:
        with (
            tc.tile_pool(name="const", bufs=1) as constp,
            tc.tile_pool(name="persist", bufs=1) as pers,
            tc.tile_pool(name="dram", bufs=1, space="DRAM") as dram,
            tc.tile_pool(name="stats", bufs=2) as statp,
            tc.tile_pool(name="vec", bufs=4) as vecp,
        ):
            ident = constp.tile([128, 128], F32, tag="ident")
            make_identity(nc, ident[:])

            # ---- load params into SBUF ----
            def load(name, shape, pool=constp):
                t = pool.tile(list(shape), F32, tag=name)
                nc.sync.dma_start(t[:], inp[name][:])
                return t

            Lt1 = load("Lt1", (4, N))
            Rt1 = load("Rt1", (4, N))
            W1T = load("W1T", (C1_IN, C1_OUT))
            W2T = load("W2T", (C1_OUT, C2_OUT))
            W3T = load("W3T", (C2_OUT, C3_OUT))
            Wg1T = load("Wg1T", (C3_OUT, G1_OUT))
            Wg2T = load("Wg2T", (G1_OUT, G2_OUT))
            W4Tp = load("W4Tp", (128, 8 * C4_OUT))
            pv = {nm: load(nm, (64, 1)) for nm in
                  ("b1", "gm1", "bt1", "b2", "gm2", "bt2", "b3", "gm3", "bt3")}
            pv.update({nm: load(nm, (128, 1)) for nm in ("bg1", "gmg1", "btg1")})
            pv.update({nm: load(nm, (128, 8)) for nm in ("bg2", "gmg2", "btg2")})
            pv.update({nm: load(nm, (128, 4)) for nm in ("b4", "gm4", "bt4")})

            ones128 = constp.tile([128, 1], F32, tag="ones128")
            nc.vector.memset(ones128[:], 1.0)
            ones_row = constp.tile([1, 128], F32, tag="ones_row")
            nc.vector.memset(ones_row[:], 1.0)

            # persistent activations
            hT = pers.tile([12, N], F32, tag="hT")
            h4 = pers.tile([64, N], F32, tag="h4")
            h5 = pers.tile([128, N], F32, tag="h5")
            mT = pers.tile([64, N], F32, tag="mT")
            m2T = pers.tile([128, N], F32, tag="m2T")

            ftbl = dram.tile([N, 64], F32, tag="ftbl")
            ft2bl = dram.tile([N, 128], F32, tag="ft2bl")

            # ---------------- BN helper ----------------
            coll_seq = [0]
            dbg_done = []

            def allreduce_stats(stats):
                """AllReduce a (128, 16) f32 stats tile across all cores."""
                i = coll_seq[0]
                coll_seq[0] += 1
                in_b = dram.tile([128, 16], F32, tag=f"arin{i}")
                out_b = dram.tile([128, 16], F32, tag=f"arout{i}")
                nc.sync.dma_start(in_b[:], stats[:])
                nc.gpsimd.collective_compute(
                    "AllReduce", OP.add,
                    replica_groups=[list(range(N_CORES))],
                    ins=[in_b.opt()], outs=[out_b.opt()],
                )
                back = statp.tile([128, 16], F32, tag="arback")
                nc.sync.dma_start(back[:], out_b[:])
                return back

            def bn_vectors(gst, col_s, col_q, gm_ap, bt_ap, c):
                """From summed stats -> (a, sh) APs of shape (c,1)."""
                v = vecp.tile([128, 8], F32, tag="bnv")
                s = gst[0:c, col_s : col_s + 1]
                q = gst[0:c, col_q : col_q + 1]
                nc.vector.tensor_scalar_mul(v[0:c, 0:1], s, INV_M)            # mu
                nc.vector.tensor_scalar_mul(v[0:c, 1:2], q, INV_M)            # E[y^2]
                nc.vector.tensor_mul(v[0:c, 2:3], v[0:c, 0:1], v[0:c, 0:1])   # mu^2
                nc.vector.tensor_sub(v[0:c, 1:2], v[0:c, 1:2], v[0:c, 2:3])   # var
                nc.vector.tensor_scalar_add(v[0:c, 1:2], v[0:c, 1:2], BN_EPS)
                nc.scalar.sqrt(v[0:c, 2:3], v[0:c, 1:2])                      # std
                nc.vector.reciprocal(v[0:c, 3:4], v[0:c, 2:3])                # 1/std
                nc.vector.tensor_mul(v[0:c, 4:5], gm_ap, v[0:c, 3:4])         # a
                nc.vector.tensor_mul(v[0:c, 5:6], v[0:c, 0:1], v[0:c, 4:5])   # mu*a
                nc.vector.tensor_sub(v[0:c, 6:7], bt_ap, v[0:c, 5:6])         # sh
                return v

            # ---------------- phase 1: knn on xyz -> covariance features ----
            with (
                tc.tile_pool(name="k1_psum", bufs=1, space="PSUM") as up,
                tc.tile_pool(name="k1_tp", bufs=2, space="PSUM") as tpp,
                tc.tile_pool(name="k1_ut", bufs=2) as utp,
                tc.tile_pool(name="k1_scr", bufs=2) as scrp,
                tc.tile_pool(name="k1_small", bufs=3) as smp,
                tc.tile_pool(name="k1_g", bufs=3) as gp,
            ):
                for i in range(NB):
                    pu = up.tile([128, N], F32, tag="u")
                    for j in range(4):
                        nc.tensor.matmul(pu[:, ts(j, 512)], Lt1[:, ts(i, 128)],
                                         Rt1[:, ts(j, 512)])
                    ut = utp.tile([128, N], F32, tag="ut")
                    nc.scalar.activation(ut[:], pu[:], AF.Identity)
                    m8 = smp.tile([128, 16], F32, tag="m8")
                    scr = scrp.tile([128, N], F32, tag="scr")
                    nc.vector.max(m8[:, 0:8], ut[:])
                    nc.vector.match_replace(scr[:], m8[:, 0:8], ut[:], NEG)
                    nc.vector.max(m8[:, 8:16], scr[:])
                    idx = smp.tile([128, 16], U32, tag="idx")
                    nc.vector.max_index(idx[:, 0:8], m8[:, 0:8], ut[:])
                    nc.vector.max_index(idx[:, 8:16], m8[:, 8:16], ut[:])

                    g = gp.tile([128, 64], F32, tag="g")
                    nc.gpsimd.indirect_dma_start(
                        g[:], None, inp["xpad"][:],
                        IndirectOffsetOnAxis(ap=idx[:], axis=0),
                    )
                    if DEBUG and i == 0:
                        nc.sync.dma_start(dbg_t[:, 0:2], ut[:, 0:2])
                        nc.sync.dma_start(dbg_t[:, 2:10], m8[:, 0:8])
                        idxf = smp.tile([128, 16], F32, tag="idxf")
                        nc.vector.tensor_copy(idxf[:], idx[:])
                        nc.sync.dma_start(dbg_t[:, 10:26], idxf[:])
                        nc.sync.dma_start(dbg_t[:, 26:34], g[:, 0:8])
                    xt = gp.tile([128, 4], F32, tag="xt")
                    nc.sync.dma_start(xt[:], inp["xpad"][ts(i, 128), :])
                    cb = smp.tile([128, 12], F32, tag="cb")
                    nc.vector.tensor_copy(cb[:, 0:3], xt[:, 0:3])
                    # mean over 16 neighbors (tree), into m[:, :4]
                    m = smp.tile([128, 32], F32, tag="mtree")
                    nc.vector.tensor_add(m[:, 0:32], g[:, 0:32], g[:, 32:64])
                    nc.vector.tensor_add(m[:, 0:16], m[:, 0:16], m[:, 16:32])
                    nc.vector.tensor_add(m[:, 0:8], m[:, 0:8], m[:, 8:16])
                    nc.vector.tensor_add(m[:, 0:4], m[:, 0:4], m[:, 4:8])
                    # scale by 1/4 so s1_c*s1_d carries 1/16
                    nc.vector.tensor_scalar_mul(m[:, 0:4], m[:, 0:4], 0.25)

                    # cq[c, d] = sum_j g[j,c]*g[j,d]  (c rows of 3)
                    pr = scrp.tile([128, 48], F32, tag="prod")
                    gjd = g[:].rearrange("p (j c) -> p j c", c=4)[:, :, 0:3]
                    for c in range(3):
                        gc = g[:, c:64:4].broadcast_to([128, 16, 3])
                        nc.vector.tensor_tensor(
                            pr[:].rearrange("p (j d) -> p j d", d=3),
                            gjd, gc, op=OP.mult)
                        nc.vector.reduce_sum(
                            cb[:, 3 + 3 * c : 6 + 3 * c],
                            pr[:].rearrange("p (j d) -> p d j", d=3), axis=AX.X)
                    # subtract s1_c*s1_d/16
                    p16 = smp.tile([128, 9], F32, tag="p16")
                    for c in range(3):
                        nc.vector.tensor_scalar_mul(
                            p16[:, 3 * c : 3 * c + 3], m[:, 0:3], m[:, c : c + 1])
                    nc.vector.tensor_sub(cb[:, 3:12], cb[:, 3:12], p16[:])
                    # transpose (128, 12) -> (12, 128) into hT
                    ptp = tpp.tile([12, 128], F32, tag="covT")
                    nc.tensor.transpose(ptp[:], cb[:], ident[:])
                    nc.scalar.activation(hT[0:12, ts(i, 128)], ptp[:], AF.Identity)

            # ---------------- phase 2: conv1..conv3 ----------------
            scr2 = pers.tile([128, N], F32, tag="sqscr")

            def conv_bn_small(rhs_ap, WT, cin, cout, b, gm, bt, h_out):
                with tc.tile_pool(name="conv_psum", bufs=1, space="PSUM") as cp:
                    py = cp.tile([cout, N], F32, tag="y")
                    for j in range(4):
                        nc.tensor.matmul(py[:, ts(j, 512)], WT[:],
                                         rhs_ap[:, ts(j, 512)])
                    stats = statp.tile([128, 16], F32, tag="st")
                    nc.vector.memset(stats[:], 0.0)
                    hpre = h_out[0:cout, :]
                    nc.scalar.activation(hpre, py[:], AF.Identity,
                                         bias=b[0:cout, 0:1],
                                         accum_out=stats[0:cout, 0:1])
                    nc.scalar.activation(scr2[0:cout, :], hpre, AF.Square,
                                         accum_out=stats[0:cout, 1:2])
                    gst = allreduce_stats(stats)
                    v = bn_vectors(gst, 0, 1, gm[0:cout, 0:1], bt[0:cout, 0:1], cout)
                    dbg_done.append(1)
                    if DEBUG and len(dbg_done) == 4:
                        nc.sync.dma_start(dbg_t[:, 38:40], stats[:, 0:2])
                        nc.sync.dma_start(dbg_t[:, 40:42], gst[:, 0:2])
                        nc.sync.dma_start(dbg_t[0:cout, 42:43], v[0:cout, 4:5])
                        nc.sync.dma_start(dbg_t[0:cout, 43:44], v[0:cout, 6:7])
                        nc.sync.dma_start(dbg_t[0:cout, 60:62], hpre[0:cout, 0:2])
                        nc.sync.dma_start(dbg_t[0:cout, 62:64], v[0:cout, 0:2])
                    nc.scalar.activation(hpre, hpre, AF.Relu,
                                         scale=v[0:cout, 4:5], bias=v[0:cout, 6:7])

            with tc.tile_pool(name="hpre_pool", bufs=2) as scrp2:
                conv_bn_small(hT[:], W1T, C1_IN, 64, pv["b1"], pv["gm1"], pv["bt1"], h4)
                if DEBUG:
                    nc.sync.dma_start(dbg_t[0:12, 34:36], hT[0:12, 0:2])
                    nc.sync.dma_start(dbg_t[0:64, 36:38], h4[0:64, 0:2])
                conv_bn_small(h4[:], W2T, 64, 64, pv["b2"], pv["gm2"], pv["bt2"], h4)
                conv_bn_small(h4[:], W3T, 64, 64, pv["b3"], pv["gm3"], pv["bt3"], h4)

                # feature table (N, 64) for g1 gather
                with tc.tile_pool(name="ft_psum", bufs=2, space="PSUM") as ftp:
                    for i in range(NB):
                        ptp = ftp.tile([128, 64], F32, tag="ftT")
                        nc.tensor.transpose(ptp[:], h4[:, ts(i, 128)],
                                            ident[0:64, 0:64])
                        ft = scrp2.tile([128, 64], F32, tag="fts")
                        nc.scalar.activation(ft[:], ptp[:], AF.Identity)
                        nc.sync.dma_start(ftbl[ts(i, 128), :], ft[:])

            # ---------------- graph layer helper ----------------
            def graph_knn(feat, cdim, ftable, pooled_T):
                """kNN in feature space + gather + max-pool; writes pooled^T
                (cdim, N) into pooled_T."""
                with tc.tile_pool(name="gk_sb", bufs=1) as sb:
                    with tc.tile_pool(name="gk_prep", bufs=1, space="PSUM") as pp:
                        # aa[n] = sum_c feat[c,n]^2  (via ones-vector matmul)
                        nc.scalar.activation(scr2[0:cdim, :], feat[:], AF.Square)
                        pa = pp.tile([1, N], F32, tag="aa")
                        for j in range(4):
                            nc.tensor.matmul(pa[:, ts(j, 512)], ones128[0:cdim, :],
                                             scr2[0:cdim, ts(j, 512)])
                        Lt = sb.tile([cdim, N], F32, tag="lt")
                        nc.scalar.activation(Lt[:], feat[:], AF.Identity, scale=2.0)
                        naa = sb.tile([1, N], F32, tag="naa")
                        nc.scalar.activation(naa[:], pa[:], AF.Identity, scale=-1.0)

                    with (
                        tc.tile_pool(name="gk_psum", bufs=1, space="PSUM") as up,
                        tc.tile_pool(name="gk_tp", bufs=2, space="PSUM") as tpp,
                        tc.tile_pool(name="gk_ut", bufs=2) as utp,
                        tc.tile_pool(name="gk_scr", bufs=2) as scrp,
                        tc.tile_pool(name="gk_small", bufs=3) as smp,
                        tc.tile_pool(name="gk_g", bufs=3) as gp,
                    ):
                        for i in range(NB):
                            pu = up.tile([128, N], F32, tag="u")
                            for j in range(4):
                                nc.tensor.matmul(pu[:, ts(j, 512)],
                                                 Lt[:, ts(i, 128)],
                                                 feat[:, ts(j, 512)],
                                                 start=True, stop=False)
                                nc.tensor.matmul(pu[:, ts(j, 512)],
                                                 ones_row[:, 0:128],
                                                 naa[:, ts(j, 512)],
                                                 start=False, stop=True)
                            ut = utp.tile([128, N], F32, tag="ut")
                            nc.scalar.activation(ut[:], pu[:], AF.Identity)
                            m8 = smp.tile([128, 16], F32, tag="m8")
                            scr = scrp.tile([128, N], F32, tag="scr")
                            nc.vector.max(m8[:, 0:8], ut[:])
                            nc.vector.match_replace(scr[:], m8[:, 0:8], ut[:], NEG)
                            nc.vector.max(m8[:, 8:16], scr[:])
                            idx = smp.tile([128, 16], U32, tag="idx")
                            nc.vector.max_index(idx[:, 0:8], m8[:, 0:8], ut[:])
                            nc.vector.max_index(idx[:, 8:16], m8[:, 8:16], ut[:])

                            g = gp.tile([128, 16 * cdim], F32, tag="g")
                            nc.gpsimd.indirect_dma_start(
                                g[:], None, ftable[:],
                                IndirectOffsetOnAxis(ap=idx[:], axis=0),
                            )
                            if DEBUG and cdim == 64 and i == 15:
                                nc.sync.dma_start(dbg2_t[:, 2048:3072], g[:, 0:1024])
                                idxf2 = smp.tile([128, 16], F32, tag="idxf2")
                                nc.vector.tensor_copy(idxf2[:], idx[:])
                                nc.sync.dma_start(dbg_t[:, 10:26], idxf2[:])
                            w = 8 * cdim
                            nc.vector.tensor_tensor(g[:, 0:w], g[:, 0:w],
                                                    g[:, w : 2 * w], op=OP.max)
                            w //= 2
                            nc.vector.tensor_tensor(g[:, 0:w], g[:, 0:w],
                                                    g[:, w : 2 * w], op=OP.max)
                            w //= 2
                            nc.vector.tensor_tensor(g[:, 0:w], g[:, 0:w],
                                                    g[:, w : 2 * w], op=OP.max)
                            w //= 2
                            nc.vector.tensor_tensor(g[:, 0:w], g[:, 0:w],
                                                    g[:, w : 2 * w], op=OP.max)
                            if DEBUG and cdim == 64 and i == 15:
                                nc.sync.dma_start(dbg2_t[:, 3072:3136], g[:, 0:64])
                            ptp = tpp.tile([cdim, 128], F32, tag="plT")
                            nc.tensor.transpose(ptp[:], g[:, 0:cdim],
                                                ident[:])
                            nc.scalar.activation(pooled_T[:, ts(i, 128)], ptp[:],
                                                 AF.Identity)

            # ---------------- phase 3: graph layer 1 ----------------
            graph_knn(h4, 64, ftbl, mT)
            if DEBUG:
                nc.sync.dma_start(dbg2_t[0:64, 0:2048], mT[:])
            with tc.tile_pool(name="hpre_pool2", bufs=2) as scrp2b:
                scrp2 = scrp2b
                conv_bn_small(mT[:], Wg1T, 64, 128, pv["bg1"], pv["gmg1"],
                              pv["btg1"], h5)
                with tc.tile_pool(name="ft2_psum", bufs=2, space="PSUM") as ftp:
                    for i in range(NB):
                        ptp = ftp.tile([128, 128], F32, tag="ft2T")
                        nc.tensor.transpose(ptp[:], h5[:, ts(i, 128)], ident[:])
                        ft = scrp2b.tile([128, 128], F32, tag="ft2s")
                        nc.scalar.activation(ft[:], ptp[:], AF.Identity)
                        nc.sync.dma_start(ft2bl[ts(i, 128), :], ft[:])

            # ---------------- phase 4: graph layer 2 + convg2 ----------------
            graph_knn(h5, 128, ft2bl, m2T)

            # convg2: (1024, 128) @ (128, N)
            latep_cm = tc.tile_pool(name="late", bufs=1)
            latep = latep_cm.__enter__()
            h6 = [latep.tile([128, N], F32, tag=f"h6_{j}", name=f"h6_{j}")
                  for j in range(8)]
            with tc.tile_pool(name="g2conv_psum", bufs=2, space="PSUM") as cp:
                stats = statp.tile([128, 16], F32, tag="stg2")
                nc.vector.memset(stats[:], 0.0)
                for mblk in range(8):
                    py = cp.tile([128, N], F32, tag="y")
                    for j in range(4):
                        nc.tensor.matmul(py[:, ts(j, 512)],
                                         Wg2T[:, ts(mblk, 128)],
                                         m2T[:, ts(j, 512)])
                    nc.scalar.activation(h6[mblk][:], py[:], AF.Identity,
                                         bias=pv["bg2"][:, mblk : mblk + 1],
                                         accum_out=stats[:, mblk : mblk + 1])
                    nc.scalar.activation(scr2[:], h6[mblk][:], AF.Square,
                                         accum_out=stats[:, 8 + mblk : 9 + mblk])
                gst = allreduce_stats(stats)
                for mblk in range(8):
                    v = bn_vectors(gst, mblk, 8 + mblk,
                                   pv["gmg2"][:, mblk : mblk + 1],
                                   pv["btg2"][:, mblk : mblk + 1], 128)
                    nc.scalar.activation(h6[mblk][:], h6[mblk][:], AF.Relu,
                                         scale=v[:, 4:5], bias=v[:, 6:7])

            # ---------------- phase 5: conv4 + BN4 + global max ----------------
            with (
                tc.tile_pool(name="c4_psum", bufs=2, space="PSUM") as cp,
                tc.tile_pool(name="c4_sb", bufs=2) as hp,
            ):
                stats = statp.tile([128, 16], F32, tag="st4")
                nc.vector.memset(stats[:], 0.0)
                maxc = pers.tile([128, 4], F32, tag="maxc")
                for mblk in range(4):
                    py = cp.tile([128, N], F32, tag="y")
                    for j in range(4):
                        for k in range(8):
                            nc.tensor.matmul(
                                py[:, ts(j, 512)],
                                W4Tp[:, 512 * k + 128 * mblk : 512 * k + 128 * mblk + 128],
                                h6[k][:, ts(j, 512)],
                                start=(k == 0), stop=(k == 7))
                    y4 = hp.tile([128, N], F32, tag="y4")
                    nc.scalar.activation(y4[:], py[:], AF.Identity,
                                         bias=pv["b4"][:, mblk : mblk + 1],
                                         accum_out=stats[:, mblk : mblk + 1])
                    nc.scalar.activation(scr2[:], y4[:], AF.Square,
                                         accum_out=stats[:, 8 + mblk : 9 + mblk])
                    nc.vector.reduce_max(maxc[:, mblk : mblk + 1], y4[:], axis=AX.X)
                gst = allreduce_stats(stats)
                out4 = pers.tile([128, 4], F32, tag="out4")
                for mblk in range(4):
                    v = bn_vectors(gst, mblk, 8 + mblk,
                                   pv["gm4"][:, mblk : mblk + 1],
                                   pv["bt4"][:, mblk : mblk + 1], 128)
                    # out = (max - mu) * a + bt  (valid since gm>0)
                    nc.vector.tensor_sub(out4[:, mblk : mblk + 1],
                                         maxc[:, mblk : mblk + 1], v[:, 0:1])
                    nc.vector.tensor_mul(out4[:, mblk : mblk + 1],
                                         out4[:, mblk : mblk + 1], v[:, 4:5])
                    nc.vector.tensor_add(out4[:, mblk : mblk + 1],
                                         out4[:, mblk : mblk + 1],
                                         pv["bt4"][:, mblk : mblk + 1])
            if DEBUG:
                nc.sync.dma_start(dbg_t[:, 44:46], h5[:, 0:2])
                nc.sync.dma_start(dbg_t[:, 46:48], h6[0][:, 0:2])
                nc.sync.dma_start(dbg_t[:, 48:52], maxc[:, 0:4])
                nc.sync.dma_start(dbg_t[:, 52:56], out4[:, 0:4])
                nc.sync.dma_start(dbg_t[0:64, 56:58], mT[0:64, 0:2])
                nc.sync.dma_start(dbg_t[:, 58:60], m2T[:, 0:2])
            with (
                tc.tile_pool(name="fin_psum", bufs=1, space="PSUM") as fp,
                tc.tile_pool(name="fin_sb", bufs=1) as fsb,
            ):
                ptp = fp.tile([4, 128], F32, tag="outT")
                nc.tensor.transpose(ptp[:], out4[:], ident[:])
                outs = fsb.tile([4, 128], F32, tag="outs")
                nc.scalar.activation(outs[:], ptp[:], AF.Identity)
                nc.sync.dma_start(out_t[:], outs[:])
            latep_cm.__exit__(None, None, None)

    split_drain_waits(nc)
    return nc


_PROGRAM = None


def _get_program():
    global _PROGRAM
    if _PROGRAM is None:
        _PROGRAM = build_program()
    return _PROGRAM


def make_in_maps(x, weights):
    """x: (B, N, 3); weights: dict of the reference param arrays."""
    shared = {}
    shared["W1T"] = np.ascontiguousarray(weights["W1"].T)
    shared["W2T"] = np.ascontiguousarray(weights["W2"].T)
    shared["W3T"] = np.ascontiguousarray(weights["W3"].T)
    shared["Wg1T"] = np.ascontiguousarray(weights["Wg1"].T)
    shared["Wg2T"] = np.ascontiguousarray(weights["Wg2"].T)
    W4 = weights["W4"]
    chunks = [np.ascontiguousarray(W4[:, 128 * j : 128 * (j + 1)].T) for j in range(8)]
    shared["W4Tp"] = np.ascontiguousarray(np.concatenate(chunks, axis=1))
    for nm in ("b1", "gm1", "bt1", "b2", "gm2", "bt2", "b3", "gm3", "bt3"):
        shared[nm] = np.ascontiguousarray(weights[nm].reshape(-1, 1))
    for nm in ("bg1", "gmg1", "btg1"):
        shared[nm] = np.ascontiguousarray(weights[nm].reshape(-1, 1))
    for nm in ("bg2", "gmg2", "btg2"):
        shared[nm] = np.ascontiguousarray(weights[nm].reshape(8, 128).T)
    for nm in ("b4", "gm4", "bt4"):
        shared[nm] = np.ascontiguousarray(weights[nm].reshape(4, 128).T)

    in_maps = []
    for c in range(B):
        xc = np.asarray(x[c], dtype=np.float32)       # (N, 3)
        xT = np.ascontiguousarray(xc.T)               # (3, N)
        aa = (xc * xc).sum(axis=1).astype(np.float32)  # (N,)
        m = dict(shared)
        m["Lt1"] = np.ascontiguousarray(
            np.concatenate([2.0 * xT, np.ones((1, N), np.float32)], axis=0))
        m["Rt1"] = np.ascontiguousarray(
            np.concatenate([xT, -aa[None, :]], axis=0))
        m["xpad"] = np.ascontiguousarray(
            np.concatenate([xc, np.zeros((N, 1), np.float32)], axis=1))
        in_maps.append(m)
    return in_maps


def kernel(**inputs):
    x = np.asarray(inputs["x"], dtype=np.float32)
    weights = {k: np.asarray(v, dtype=np.float32)
               for k, v in inputs.items() if k != "x"}
    nc = _get_program()
    in_maps = make_in_maps(x, weights)
    res = run_bass_kernel_spmd(nc, in_maps, core_ids=list(range(N_CORES)),
                               trace=False)
    out = np.stack([res.results[c]["out"].reshape(512) for c in range(B)])
    return out.astype(np.float32)


if __name__ == "__main__":
    nc = build_program()
    print("program built ok")

